# revision 1
# baseline (speedup 1.0000x reference)
"""MoE MLP (E=32 experts, top-2, D=H=1024) on 8 Trainium2 NeuronCores.

Strategy (expert parallel, per sharding hint):
  * Host computes the (tiny) gate: softmax(x @ Wg), top-2, renormalized
    weights. This is the dispatch/sharding step.
  * Tokens are gathered per expert into capacity-padded blocks, transposed
    to [D, tokens] so that on device features live on SBUF partitions and
    tokens are the matmul moving/free dimension.
  * Each of the 8 cores owns 4 experts (W1/W2/b1/b2 shards) and runs
    GELU(x W1 + b1) W2 + b2 for its experts' token blocks.
  * Host combines with the top-2 gate weights (scatter-add).

The device kernel is a fully-unrolled Tile program: per expert, stream
W1[e]/W2[e] (4MB each fp32) into SBUF, 8x8 128-tiles of matmuls per layer
accumulating over the contraction dim in PSUM, fused bias+GELU on the
scalar engine, bias-add epilogue on the vector engine.
"""

import os
import sys
import numpy as np

for _p in ("/root/.axon_site/_ro/trn_rl_repo", "/opt/trn_rl_repo"):
    if _p not in sys.path and os.path.isdir(_p):
        sys.path.append(_p)

E, D, H = 32, 1024, 1024
TOP_K = 2
N_CORES = 8
EPC = E // N_CORES  # experts per core
ND = D // 128       # d 128-tiles
NH = H // 128       # h 128-tiles

# matmul dtype: "float32" (exact, 4 cyc/row), "float32r" (fast fp32, 1 cyc/row
# at N>=256), "bfloat16" (fast + halves weight DMA)
MM_DTYPE = os.environ.get("MOE_MM_DTYPE", "float32r")

LAST_EXEC_TIME_NS = None

_NC_CACHE = {}


def _build_nc(TCH, CW, mm_dtype):
    import concourse.bass as bass  # noqa: F401
    import concourse.tile as tile
    from concourse import bacc, mybir
    from contextlib import ExitStack

    f32 = mybir.dt.float32
    dt_mm = getattr(mybir.dt, mm_dtype)
    C = TCH * CW

    nc = bacc.Bacc(
        "TRN2",
        target_bir_lowering=False,
        debug=False,
        enable_asserts=False,
        num_devices=N_CORES,
    )
    xT = nc.dram_tensor("xT", [D, EPC * C], dt_mm, kind="ExternalInput").ap()
    w1 = nc.dram_tensor("w1", [EPC, D, H], dt_mm, kind="ExternalInput").ap()
    w2 = nc.dram_tensor("w2", [EPC, H, D], dt_mm, kind="ExternalInput").ap()
    b1 = nc.dram_tensor("b1", [EPC, H], f32, kind="ExternalInput").ap()
    b2 = nc.dram_tensor("b2", [EPC, D], f32, kind="ExternalInput").ap()
    yT = nc.dram_tensor("yT", [D, EPC * C], f32, kind="ExternalOutput").ap()

    with tile.TileContext(nc) as tc, ExitStack() as ctx:
        wpool = ctx.enter_context(tc.tile_pool(name="w", bufs=2))
        xpool = ctx.enter_context(tc.tile_pool(name="x", bufs=2 * ND))
        hpool = ctx.enter_context(tc.tile_pool(name="h", bufs=2 * NH))
        ypool = ctx.enter_context(tc.tile_pool(name="y", bufs=4))
        bpool = ctx.enter_context(tc.tile_pool(name="b", bufs=1))
        pp1 = ctx.enter_context(tc.tile_pool(name="ps1", bufs=2, space="PSUM"))
        pp2 = ctx.enter_context(tc.tile_pool(name="ps2", bufs=2, space="PSUM"))

        b1_sb = bpool.tile([128, EPC * NH], f32, tag="b1")
        b2_sb = bpool.tile([128, EPC * ND], f32, tag="b2")
        for e in range(EPC):
            nc.sync.dma_start(
                out=b1_sb[:, e * NH:(e + 1) * NH],
                in_=b1[e].rearrange("(ht p) -> p ht", p=128),
            )
            nc.sync.dma_start(
                out=b2_sb[:, e * ND:(e + 1) * ND],
                in_=b2[e].rearrange("(dt p) -> p dt", p=128),
            )

        gelu = mybir.ActivationFunctionType.Gelu
        for e in range(EPC):
            # W1[e]: SBUF [p=d_in, (d_t, h)]; lhsT tile (d_t,h_t) is a slice
            w1_sb = wpool.tile([128, ND * H], dt_mm, tag="w1")
            nc.sync.dma_start(
                out=w1_sb[:].rearrange("p (dt h) -> p dt h", dt=ND),
                in_=w1[e].rearrange("(dt p) h -> p dt h", p=128),
            )
            xts = []
            for dt_i in range(ND):
                xt = xpool.tile([128, C], dt_mm, tag="xt")
                nc.sync.dma_start(
                    out=xt[:],
                    in_=xT[dt_i * 128:(dt_i + 1) * 128, e * C:(e + 1) * C],
                )
                xts.append(xt)
            w2_sb = wpool.tile([128, NH * D], dt_mm, tag="w2")
            nc.sync.dma_start(
                out=w2_sb[:].rearrange("p (ht d) -> p ht d", ht=NH),
                in_=w2[e].rearrange("(ht p) d -> p ht d", p=128),
            )

            for ch in range(TCH):
                hts = []
                for ht in range(NH):
                    ps = pp1.tile([128, CW], f32, tag="ps1")
                    for dt_i in range(ND):
                        nc.tensor.matmul(
                            ps[:],
                            w1_sb[:, dt_i * H + ht * 128: dt_i * H + ht * 128 + 128],
                            xts[dt_i][:, ch * CW:(ch + 1) * CW],
                            start=(dt_i == 0),
                            stop=(dt_i == ND - 1),
                        )
                    hsb = hpool.tile([128, CW], dt_mm, tag="ht")
                    nc.scalar.activation(
                        hsb[:], ps[:], gelu,
                        bias=b1_sb[:, e * NH + ht: e * NH + ht + 1],
                    )
                    hts.append(hsb)
                for dt_i in range(ND):
                    ps2 = pp2.tile([128, CW], f32, tag="ps2")
                    for ht in range(NH):
                        nc.tensor.matmul(
                            ps2[:],
                            w2_sb[:, ht * D + dt_i * 128: ht * D + dt_i * 128 + 128],
                            hts[ht][:],
                            start=(ht == 0),
                            stop=(ht == NH - 1),
                        )
                    ysb = ypool.tile([128, CW], f32, tag="yt")
                    nc.vector.tensor_scalar_add(
                        ysb[:], ps2[:],
                        b2_sb[:, e * ND + dt_i: e * ND + dt_i + 1],
                    )
                    nc.sync.dma_start(
                        out=yT[dt_i * 128:(dt_i + 1) * 128,
                               e * C + ch * CW: e * C + (ch + 1) * CW],
                        in_=ysb[:],
                    )
    nc.compile()
    return nc


def _get_nc(TCH, CW, mm_dtype):
    key = (TCH, CW, mm_dtype)
    if key not in _NC_CACHE:
        _NC_CACHE[key] = _build_nc(TCH, CW, mm_dtype)
    return _NC_CACHE[key]


def _route(xf, Wg):
    """Replicates the reference gate exactly in f32 numpy."""
    logits = xf @ Wg                                     # [T, E]
    m = logits.max(-1, keepdims=True)
    ex = np.exp(logits - m)
    scores = ex / ex.sum(-1, keepdims=True)
    idx = np.argsort(-scores, axis=1, kind="stable")[:, :TOP_K]  # [T, k]
    tw = np.take_along_axis(scores, idx, 1)
    m2 = tw.max(-1, keepdims=True)
    e2 = np.exp(tw - m2)
    w = (e2 / e2.sum(-1, keepdims=True)).astype(np.float32)
    return idx.astype(np.int64), w


def kernel(x, Wg, W1, b1, W2, b2):
    global LAST_EXEC_TIME_NS
    from concourse import bass_utils

    mm_dtype = MM_DTYPE
    orig_shape = x.shape
    x = np.asarray(x, dtype=np.float32)
    Wg = np.asarray(Wg, dtype=np.float32)
    W1 = np.asarray(W1, dtype=np.float32)
    b1 = np.asarray(b1, dtype=np.float32)
    W2 = np.asarray(W2, dtype=np.float32)
    b2 = np.asarray(b2, dtype=np.float32)
    xf = np.ascontiguousarray(x.reshape(-1, D))
    T = xf.shape[0]

    idx, w = _route(xf, Wg)

    # ---- dispatch: per-expert capacity-padded token blocks
    flat_e = idx.reshape(-1)                 # [k*T]
    flat_t = np.repeat(np.arange(T), TOP_K)
    order = np.argsort(flat_e, kind="stable")
    counts = np.bincount(flat_e, minlength=E)
    maxc = int(counts.max())
    C = max(256, -(-maxc // 64) * 64)
    TCH = -(-C // 512)
    CW = -(-C // (TCH * 64)) * 64
    C = TCH * CW

    starts = np.zeros(E + 1, np.int64)
    starts[1:] = np.cumsum(counts)
    se = flat_e[order]
    pos = np.arange(TOP_K * T) - starts[se]
    core = se // EPC
    col = (se % EPC) * C + pos               # column in that core's xT
    tok = flat_t[order]

    gidx = np.zeros((N_CORES, EPC * C), np.int64)
    for c in range(N_CORES):
        msel = core == c
        gidx[c, col[msel]] = tok[msel]

    if mm_dtype == "bfloat16":
        import ml_dtypes
        np_mm = np.dtype(ml_dtypes.bfloat16)
    else:
        np_mm = np.dtype(np.float32)
    xf_mm = xf.astype(np_mm, copy=False)
    W1_mm = W1.astype(np_mm, copy=False)
    W2_mm = W2.astype(np_mm, copy=False)

    in_maps = []
    for c in range(N_CORES):
        e0 = c * EPC
        in_maps.append({
            "xT": np.ascontiguousarray(xf_mm[gidx[c]].T),
            "w1": W1_mm[e0:e0 + EPC],
            "w2": W2_mm[e0:e0 + EPC],
            "b1": b1[e0:e0 + EPC],
            "b2": b2[e0:e0 + EPC],
        })

    nc = _get_nc(TCH, CW, mm_dtype)
    trace = os.environ.get("MOE_TRACE", "0") == "1"
    res = bass_utils.run_bass_kernel_spmd(
        nc, in_maps, core_ids=list(range(N_CORES)), trace=trace,
    )
    LAST_EXEC_TIME_NS = res.exec_time_ns

    # ---- combine: gather each (token, k) contribution, weight, and sum
    Ystack = np.stack([res.results[c]["yT"].T for c in range(N_CORES)])
    contrib = Ystack[core, col]              # [k*T, D] (sorted order)
    inv = np.empty_like(order)
    inv[order] = np.arange(TOP_K * T)
    contrib = contrib[inv].reshape(T, TOP_K, D)
    y = (contrib * w[:, :, None]).sum(1).astype(np.float32)
    return y.reshape(orig_shape)
